# revision 2
# baseline (speedup 1.0000x reference)
"""Trainium2 Bass kernel v4 for the LogicMessagePassingNetwork problem.

Reference computation (E=1M edges, T=2M triangles, R=50, D=64):
    x   = edge_feat + relation_emb[edge_rel]                      # [E, D]
    m   = relu((x[edge_ab] * x[edge_bc]) @ W_msg)                 # [T, D]
    agg = segment_sum(m, edge_ac, E)                              # [E, D]
    out = relu(x + agg @ W_upd)                                   # [E, D]

v3 = v2 (batched indirect gathers, host-precomputed x table, sharded by
aggregation target) plus:
  - per-block fused PSUM tiles: one prodT copy [64, TB*128] and one
    m-relu [128, TB*64] per block instead of per chunk (ACT was 72% busy)
  - one combined [128, 3G] idx+acrel load per group instead of 3 DMAs
  - partition-major DRAM layouts for xown/out so epilogue DMAs have 2KB
    contiguous runs (128 descriptors instead of 1024)
  - optional bf16 stages (gather table, DVE elementwise, PE matmuls) via
    CFG; PSUM accumulation stays fp32
"""
import numpy as np

E = 1_000_000
T = 2_000_000
R = 50
D = 64
NCORES = 8
EPC = E // NCORES          # edges per core
BLK = 128                  # output edges per block
NBLK = (EPC + BLK - 1) // BLK          # 977 blocks/core
EPAD = NBLK * BLK                      # padded edges/core (125056)
G = 32                     # chunks per gather group
EB = 8                     # blocks per epilogue batch
ZROW = E                   # index of the all-zero row in the x table
XROWS = E + 1

# dtype knobs: "f32" or "bf16"
CFG = dict(
    xt="f32",       # gather table + gathers + prod mul inputs
    prod="f32",     # prod tile (transpose input), prodT
    mm="f32",       # wmsg/m_sb/oh/scatter + wupd/aggT/ident/xo matmul dtype
)


# ----------------------------------------------------------------- host prep
def host_preprocess(edge_rel, edge_ab, edge_bc, edge_ac, tb_override=None):
    """Index-space preprocessing. Returns per-core index arrays + TB."""
    edge_rel = np.asarray(edge_rel).astype(np.int64)
    ab = np.asarray(edge_ab).astype(np.int64)
    bc = np.asarray(edge_bc).astype(np.int64)
    ac = np.asarray(edge_ac).astype(np.int64)

    order = np.argsort(ac, kind="stable")
    ab_s, bc_s, ac_s = ab[order], bc[order], ac[order]

    per_core = []
    max_cnt = 0
    for k in range(NCORES):
        lo, hi = np.searchsorted(ac_s, [k * EPC, (k + 1) * EPC])
        c_ab, c_bc, c_ac = ab_s[lo:hi], bc_s[lo:hi], ac_s[lo:hi] - k * EPC
        ccnt = np.bincount(c_ac // BLK, minlength=NBLK)
        max_cnt = max(max_cnt, int(ccnt.max()) if len(ccnt) else 0)
        per_core.append((c_ab, c_bc, c_ac, ccnt))

    TB = tb_override or -(-max_cnt // 128)      # chunks per block
    NCHUNK = NBLK * TB
    NGRP = -(-NCHUNK // G)
    NCHUNKP = NGRP * G
    NT = NCHUNKP * 128                          # padded triangle slots/core

    outs = []
    for k in range(NCORES):
        c_ab, c_bc, c_ac, ccnt = per_core[k]
        starts = np.zeros(NBLK, np.int64)
        starts[1:] = np.cumsum(ccnt)[:-1]
        rank = np.arange(len(c_ac)) - starts[c_ac // BLK]
        slot = (c_ac // BLK) * (TB * 128) + rank

        gab = np.full(NT, ZROW, np.int32)
        gbc = np.full(NT, ZROW, np.int32)
        acrel = np.full(NT, 999.0, np.float32)
        gab[slot] = c_ab
        gbc[slot] = c_bc
        acrel[slot] = (c_ac % BLK).astype(np.float32)

        # combined [NGRP, 128 slots, 3G]: cols [0,G)=ab, [G,2G)=bc,
        # [2G,3G)=acrel-as-f32-bits
        gab = gab.reshape(NGRP, G, 128).transpose(0, 2, 1)
        gbc = gbc.reshape(NGRP, G, 128).transpose(0, 2, 1)
        acrel = acrel.reshape(NGRP, G, 128).transpose(0, 2, 1)
        comb = np.concatenate(
            [gab, gbc, acrel.view(np.int32)], axis=2)
        outs.append(dict(idx=np.ascontiguousarray(comb)))
    return outs, TB, NGRP


def build_xtable(edge_feat, relation_emb, edge_rel, np_dt):
    """x = edge_feat + relation_emb[edge_rel], plus zero row at ZROW."""
    xt = np.zeros((XROWS, D), np.float32)
    xt[:E] = np.asarray(edge_feat, np.float32) \
        + np.asarray(relation_emb, np.float32)[np.asarray(edge_rel).astype(np.int64)]
    return xt.astype(np_dt)


def build_xown(xt32, np_dt, nblk):
    """Per-core own-edge x rows, partition-major [128, nblk, D]."""
    xo = []
    for k in range(NCORES):
        rows = np.zeros((EPAD, D), np.float32)
        n = min(EPC, E - k * EPC)
        rows[:n] = xt32[k * EPC:k * EPC + n]
        # edge e = b*128 + p  ->  [p, b, :]
        pm = rows.reshape(NBLK, BLK, D).transpose(1, 0, 2)[:, :nblk]
        xo.append(np.ascontiguousarray(pm).astype(np_dt))
    return xo


# ------------------------------------------------------------- device kernel
def build_bass(TB, nblk, ngrp=None):
    import concourse.bass as bass
    import concourse.bacc as bacc
    import concourse.mybir as mybir
    import concourse.tile as tile
    from concourse.masks import make_identity

    f32 = mybir.dt.float32
    i32 = mybir.dt.int32
    dt_xt = getattr(mybir.dt, {"f32": "float32", "bf16": "bfloat16"}[CFG["xt"]])
    dt_pr = getattr(mybir.dt, {"f32": "float32", "bf16": "bfloat16"}[CFG["prod"]])
    dt_mm = getattr(mybir.dt, {"f32": "float32", "bf16": "bfloat16"}[CFG["mm"]])
    nchunk = nblk * TB
    if ngrp is None:
        ngrp = -(-nchunk // G)
    nc = bacc.Bacc(None, target_bir_lowering=False)

    xt = nc.dram_tensor("xt", [XROWS, D], dt_xt, kind="ExternalInput")
    xown = nc.dram_tensor("xown", [128, nblk, D], dt_mm, kind="ExternalInput")
    wmsg = nc.dram_tensor("wmsg", [D, D], dt_mm, kind="ExternalInput")
    wupd = nc.dram_tensor("wupd", [D, D], dt_mm, kind="ExternalInput")
    iota = nc.dram_tensor("iota", [128, 128], f32, kind="ExternalInput")
    idx = nc.dram_tensor("idx", [ngrp, 128, 3 * G], i32, kind="ExternalInput")
    out = nc.dram_tensor("out", [128, nblk, D], f32, kind="ExternalOutput")

    with tile.TileContext(nc) as tc:
        with tc.tile_pool(name="const", bufs=1) as cpool, \
             tc.tile_pool(name="gath", bufs=3) as gpool, \
             tc.tile_pool(name="idxp", bufs=3) as ipool, \
             tc.tile_pool(name="work", bufs=4) as wpool, \
             tc.tile_pool(name="outp", bufs=2) as opool, \
             tc.tile_pool(name="pst", bufs=2, space="PSUM") as pstpool, \
             tc.tile_pool(name="psm", bufs=2, space="PSUM") as psmpool, \
             tc.tile_pool(name="psagg", bufs=2, space="PSUM") as paggpool, \
             tc.tile_pool(name="psupd", bufs=2, space="PSUM") as pupdpool:

            wmsg_sb = cpool.tile([D, D], dt_mm)
            nc.sync.dma_start(out=wmsg_sb[:], in_=wmsg[:])
            wupd_sb = cpool.tile([D, D], dt_mm)
            nc.sync.dma_start(out=wupd_sb[:], in_=wupd[:])
            iota_sb = cpool.tile([128, 128], f32)
            nc.sync.dma_start(out=iota_sb[:], in_=iota[:])
            identf = cpool.tile([128, 128], f32)
            make_identity(nc, identf[:])
            if dt_mm != f32:
                ident = cpool.tile([128, 128], dt_mm)
                nc.vector.tensor_copy(out=ident[:], in_=identf[:])
            else:
                ident = identf

            cur = {}

            def load_group(g):
                ix = ipool.tile([128, 3 * G], i32, tag="ix")
                nc.sync.dma_start(out=ix[:], in_=idx[g])
                xa = gpool.tile([128, G * D], dt_xt, tag="xa")
                xb = gpool.tile([128, G * D], dt_xt, tag="xb")
                for j in range(G):
                    nc.gpsimd.indirect_dma_start(
                        out=xa[:, j * D:(j + 1) * D], out_offset=None, in_=xt[:],
                        in_offset=bass.IndirectOffsetOnAxis(ap=ix[:, j:j + 1],
                                                            axis=0))
                    nc.gpsimd.indirect_dma_start(
                        out=xb[:, j * D:(j + 1) * D], out_offset=None, in_=xt[:],
                        in_offset=bass.IndirectOffsetOnAxis(ap=ix[:, G + j:G + j + 1],
                                                            axis=0))
                cur["xa"], cur["xb"], cur["ix"] = xa, xb, ix

            xo8 = None
            ob8 = None

            for b in range(nblk):
                # ---- per-block fused DVE stages (may straddle group bdry) --
                prod3 = wpool.tile([128, TB * D], dt_pr, tag="prod3")
                oh3 = wpool.tile([128, TB * 128], dt_mm, tag="oh3")
                c = 0
                while c < TB:
                    ch = b * TB + c
                    g, j = divmod(ch, G)
                    if cur.get("g") != g:
                        load_group(g)
                        cur["g"] = g
                    xa, xb, ix = cur["xa"], cur["xb"], cur["ix"]
                    run = min(TB - c, G - j)   # chunks in this group run
                    nc.vector.tensor_mul(
                        out=prod3[:, c * D:(c + run) * D],
                        in0=xa[:, j * D:(j + run) * D],
                        in1=xb[:, j * D:(j + run) * D])
                    ar = ix[:, 2 * G + j:2 * G + j + run].bitcast(f32)
                    ar_b = bass.AP(ar.tensor, ar.offset,
                                   [ar.ap[0], ar.ap[1], (0, 128)])
                    io_ap = iota_sb[:]
                    io_b = bass.AP(io_ap.tensor, io_ap.offset,
                                   [io_ap.ap[0], (0, run), io_ap.ap[1]])
                    oh_out = oh3[:, c * 128:(c + run) * 128]
                    nc.vector.tensor_tensor(
                        out=oh_out.rearrange("p (c e) -> p c e", c=run),
                        in0=ar_b, in1=io_b, op=mybir.AluOpType.is_equal)
                    c += run

                # ---- PE transposes into one PSUM tile, one copy ----
                prodT_ps = pstpool.tile([D, TB * 128], f32, space="PSUM",
                                        tag="prodT")
                for c in range(TB):
                    nc.tensor.transpose(out=prodT_ps[:, c * 128:(c + 1) * 128],
                                        in_=prod3[:, c * D:(c + 1) * D],
                                        identity=ident[:])
                prodT = wpool.tile([D, TB * 128], dt_mm, tag="prodTs")
                nc.scalar.activation(out=prodT[:], in_=prodT_ps[:],
                                     func=mybir.ActivationFunctionType.Copy)

                # ---- W_msg matmuls into one PSUM tile, one relu ----
                m_ps = psmpool.tile([128, TB * D], f32, space="PSUM", tag="mps")
                for c in range(TB):
                    nc.tensor.matmul(out=m_ps[:, c * D:(c + 1) * D],
                                     lhsT=prodT[:, c * 128:(c + 1) * 128],
                                     rhs=wmsg_sb[:], start=True, stop=True)
                m_sb = wpool.tile([128, TB * D], dt_mm, tag="msb")
                if b % 2 == 0:
                    nc.scalar.activation(out=m_sb[:], in_=m_ps[:],
                                         func=mybir.ActivationFunctionType.Relu)
                else:
                    nc.vector.tensor_scalar(out=m_sb[:], in0=m_ps[:],
                                            scalar1=0.0, scalar2=None,
                                            op0=mybir.AluOpType.max)

                # ---- scatter matmuls accumulate aggT ----
                aggT = paggpool.tile([D, 128], f32, space="PSUM", tag="aggT")
                for c in range(TB):
                    nc.tensor.matmul(out=aggT[:], lhsT=m_sb[:, c * D:(c + 1) * D],
                                     rhs=oh3[:, c * 128:(c + 1) * 128],
                                     start=(c == 0), stop=(c == TB - 1))

                # ---- block epilogue ----
                bb = b % EB
                if bb == 0:
                    nb = min(EB, nblk - b)
                    xo8 = opool.tile([128, EB * D], dt_mm, tag="xo8")
                    nc.sync.dma_start(out=xo8[:, :nb * D], in_=xown[:, b:b + nb])
                    ob8 = opool.tile([128, EB * D], f32, tag="ob8")

                aggT_sb = wpool.tile([D, 128], dt_mm, tag="aggTs")
                nc.vector.tensor_copy(out=aggT_sb[:], in_=aggT[:])
                upd_ps = pupdpool.tile([128, D], f32, space="PSUM", tag="upd")
                nc.tensor.matmul(out=upd_ps[:], lhsT=aggT_sb[:], rhs=wupd_sb[:],
                                 start=True, stop=False)
                nc.tensor.matmul(out=upd_ps[:], lhsT=ident[:],
                                 rhs=xo8[:, bb * D:(bb + 1) * D],
                                 start=False, stop=True)
                nc.scalar.activation(out=ob8[:, bb * D:(bb + 1) * D], in_=upd_ps[:],
                                     func=mybir.ActivationFunctionType.Relu)

                if bb == EB - 1 or b == nblk - 1:
                    nb = bb + 1
                    b0 = b - bb
                    nc.sync.dma_start(out=out[:, b0:b0 + nb],
                                      in_=ob8[:, :nb * D])

    nc.compile()
    return nc


# ------------------------------------------------------------------ helpers
def np_dt(key):
    import ml_dtypes
    return {"f32": np.float32, "bf16": ml_dtypes.bfloat16}[CFG[key]]


def make_in_maps(inputs, pre, nblk, ngrp):
    xt32 = build_xtable(inputs["edge_feat"], inputs["relation_emb"],
                        inputs["edge_rel"], np.float32)
    xt = xt32.astype(np_dt("xt"))
    xos = build_xown(xt32, np_dt("mm"), nblk)
    iota = np.tile(np.arange(128, dtype=np.float32), (128, 1))
    in_maps = []
    for k in range(NCORES):
        in_maps.append({
            "xt": xt,
            "xown": xos[k],
            "wmsg": np.asarray(inputs["W_msg"], np.float32).astype(np_dt("mm")),
            "wupd": np.asarray(inputs["W_upd"], np.float32).astype(np_dt("mm")),
            "iota": iota,
            "idx": pre[k]["idx"][:ngrp],
        })
    return in_maps


def unshard_out(core_outs, nblk):
    """core_outs[k]: [128, nblk, D] partition-major -> full [E, D]."""
    full = np.empty((E, D), np.float32)
    for k in range(NCORES):
        pm = core_outs[k]          # [128, nblk, D]
        rows = pm.transpose(1, 0, 2).reshape(-1, D)    # edge e = b*128+p
        n = min(EPC, E - k * EPC)
        full[k * EPC:k * EPC + n] = rows[:n]
    return full


def run_full(inputs, nblk=NBLK):
    from concourse.bass_utils import run_bass_kernel_spmd
    pre, TB, NGRP = host_preprocess(inputs["edge_rel"], inputs["edge_ab"],
                                    inputs["edge_bc"], inputs["edge_ac"])
    nchunk = nblk * TB
    ngrp = -(-nchunk // G) if nblk < NBLK else NGRP
    import time as _time
    t0 = _time.time()
    nc = build_bass(TB, nblk, ngrp)
    print(f"[build+compile {_time.time()-t0:.1f}s TB={TB}]", flush=True)
    in_maps = make_in_maps(inputs, pre, nblk, ngrp)
    t0 = _time.time()
    res = run_bass_kernel_spmd(nc, in_maps, core_ids=list(range(NCORES)))
    print(f"[run1 {_time.time()-t0:.1f}s]", flush=True)
    outs = [np.asarray(res.results[k]["out"]) for k in range(NCORES)]
    return unshard_out(outs, nblk)


# ------------------------------------------------------------------ entry
def kernel(**inputs):
    """Full unsharded inputs -> full [E, D] output (8-core SPMD)."""
    out = run_full(inputs, nblk=NBLK)
    return out.astype(np.float32)


# revision 3
# speedup vs baseline: 1.0539x; 1.0539x over previous
"""Trainium2 Bass kernel v5 for the LogicMessagePassingNetwork problem.

Reference computation (E=1M edges, T=2M triangles, R=50, D=64):
    x   = edge_feat + relation_emb[edge_rel]                      # [E, D]
    m   = relu((x[edge_ab] * x[edge_bc]) @ W_msg)                 # [T, D]
    agg = segment_sum(m, edge_ac, E)                              # [E, D]
    out = relu(x + agg @ W_upd)                                   # [E, D]

v5 = v4 plus W=4-block scatter windows: triangles are bucketed per
512-edge window instead of per 128-edge block, cutting slot padding from
~46% to ~12% and with it the number of [128,1]-offset indirect gathers
(the SWDGE ~1us/call serial bottleneck). Each chunk scatters via 4
matmuls against a [128, 512] one-hot into a single-bank [64, 512] PSUM
accumulator holding all 4 blocks of the window.
"""
import numpy as np

E = 1_000_000
T = 2_000_000
R = 50
D = 64
NCORES = 8
EPC = E // NCORES          # edges per core
BLK = 128                  # output edges per block
W = 4                      # blocks per scatter window
WE = W * BLK               # edges per window (512)
NW = (EPC + WE - 1) // WE              # 245 windows/core
NBLK = NW * W                          # 980 blocks/core (padded)
EPAD = NBLK * BLK                      # padded edges/core (125440)
G = 32                     # chunks per idx-load group
EB = 8                     # blocks per epilogue batch (= 2 windows)
ZROW = E                   # index of the all-zero row in the x table
XROWS = E + 1
MS = 3                     # chunks fused per prodT/m_ps PSUM tile


# ----------------------------------------------------------------- host prep
def host_preprocess(edge_rel, edge_ab, edge_bc, edge_ac, tb_override=None):
    """Index-space preprocessing. Returns per-core index arrays + TBW + ngrp."""
    ab = np.asarray(edge_ab).astype(np.int64)
    bc = np.asarray(edge_bc).astype(np.int64)
    ac = np.asarray(edge_ac).astype(np.int64)

    order = np.argsort(ac, kind="stable")
    ab_s, bc_s, ac_s = ab[order], bc[order], ac[order]

    per_core = []
    max_cnt = 0
    for k in range(NCORES):
        lo, hi = np.searchsorted(ac_s, [k * EPC, (k + 1) * EPC])
        c_ab, c_bc, c_ac = ab_s[lo:hi], bc_s[lo:hi], ac_s[lo:hi] - k * EPC
        ccnt = np.bincount(c_ac // WE, minlength=NW)
        max_cnt = max(max_cnt, int(ccnt.max()) if len(ccnt) else 0)
        per_core.append((c_ab, c_bc, c_ac, ccnt))

    TBW = tb_override or -(-max_cnt // 128)     # chunks per window
    NCHUNK = NW * TBW
    NGRP = -(-NCHUNK // G)
    NT = NGRP * G * 128                         # padded triangle slots/core

    outs = []
    for k in range(NCORES):
        c_ab, c_bc, c_ac, ccnt = per_core[k]
        starts = np.zeros(NW, np.int64)
        starts[1:] = np.cumsum(ccnt)[:-1]
        rank = np.arange(len(c_ac)) - starts[c_ac // WE]
        slot = (c_ac // WE) * (TBW * 128) + rank

        gab = np.full(NT, ZROW, np.int32)
        gbc = np.full(NT, ZROW, np.int32)
        acrel = np.full(NT, 9999.0, np.float32)
        gab[slot] = c_ab
        gbc[slot] = c_bc
        acrel[slot] = (c_ac % WE).astype(np.float32)

        gab = gab.reshape(NGRP, G, 128).transpose(0, 2, 1)
        gbc = gbc.reshape(NGRP, G, 128).transpose(0, 2, 1)
        acrel = acrel.reshape(NGRP, G, 128).transpose(0, 2, 1)
        comb = np.concatenate([gab, gbc, acrel.view(np.int32)], axis=2)
        outs.append(dict(idx=np.ascontiguousarray(comb)))
    return outs, TBW, NGRP


def build_xtable(edge_feat, relation_emb, edge_rel):
    xt = np.zeros((XROWS, D), np.float32)
    xt[:E] = np.asarray(edge_feat, np.float32) \
        + np.asarray(relation_emb, np.float32)[np.asarray(edge_rel).astype(np.int64)]
    return xt


def build_xown(xt32, nblk):
    """Per-core own-edge x rows, partition-major [128, nblk, D]."""
    xo = []
    for k in range(NCORES):
        rows = np.zeros((EPAD, D), np.float32)
        n = min(EPC, E - k * EPC)
        rows[:n] = xt32[k * EPC:k * EPC + n]
        pm = rows.reshape(NBLK, BLK, D).transpose(1, 0, 2)[:, :nblk]
        xo.append(np.ascontiguousarray(pm))
    return xo


# ------------------------------------------------------------- device kernel
def build_bass(TBW, nw, ngrp=None):
    """nw = number of windows to emit (< NW for scaled-down testing)."""
    import concourse.bass as bass
    import concourse.bacc as bacc
    import concourse.mybir as mybir
    import concourse.tile as tile
    from concourse.masks import make_identity

    f32 = mybir.dt.float32
    i32 = mybir.dt.int32
    nblk = nw * W
    nchunk = nw * TBW
    if ngrp is None:
        ngrp = -(-nchunk // G)
    nc = bacc.Bacc(None, target_bir_lowering=False)

    xt = nc.dram_tensor("xt", [XROWS, D], f32, kind="ExternalInput")
    xown = nc.dram_tensor("xown", [128, nblk, D], f32, kind="ExternalInput")
    wmsg = nc.dram_tensor("wmsg", [D, D], f32, kind="ExternalInput")
    wupd = nc.dram_tensor("wupd", [D, D], f32, kind="ExternalInput")
    iotaw = nc.dram_tensor("iotaw", [128, WE], f32, kind="ExternalInput")
    idx = nc.dram_tensor("idx", [ngrp, 128, 3 * G], i32, kind="ExternalInput")
    out = nc.dram_tensor("out", [128, nblk, D], f32, kind="ExternalOutput")

    with tile.TileContext(nc) as tc:
        with tc.tile_pool(name="const", bufs=1) as cpool, \
             tc.tile_pool(name="gath", bufs=3) as gpool, \
             tc.tile_pool(name="idxp", bufs=3) as ipool, \
             tc.tile_pool(name="work", bufs=6) as wpool, \
             tc.tile_pool(name="ohp", bufs=2) as ohpool, \
             tc.tile_pool(name="outp", bufs=2) as opool, \
             tc.tile_pool(name="pst", bufs=2, space="PSUM") as pstpool, \
             tc.tile_pool(name="psm", bufs=2, space="PSUM") as psmpool, \
             tc.tile_pool(name="psagg", bufs=2, space="PSUM") as paggpool, \
             tc.tile_pool(name="psupd", bufs=2, space="PSUM") as pupdpool:

            wmsg_sb = cpool.tile([D, D], f32)
            nc.sync.dma_start(out=wmsg_sb[:], in_=wmsg[:])
            wupd_sb = cpool.tile([D, D], f32)
            nc.sync.dma_start(out=wupd_sb[:], in_=wupd[:])
            iota_sb = cpool.tile([128, WE], f32)
            nc.sync.dma_start(out=iota_sb[:], in_=iotaw[:])
            ident = cpool.tile([128, 128], f32)
            make_identity(nc, ident[:])

            cur = {"g": -1}

            def load_group(g):
                ix = ipool.tile([128, 3 * G], i32, tag="ix")
                nc.sync.dma_start(out=ix[:], in_=idx[g])
                xa = gpool.tile([128, G * D], f32, tag="xa")
                xb = gpool.tile([128, G * D], f32, tag="xb")
                for j in range(G):
                    nc.gpsimd.indirect_dma_start(
                        out=xa[:, j * D:(j + 1) * D], out_offset=None, in_=xt[:],
                        in_offset=bass.IndirectOffsetOnAxis(ap=ix[:, j:j + 1],
                                                            axis=0))
                    nc.gpsimd.indirect_dma_start(
                        out=xb[:, j * D:(j + 1) * D], out_offset=None, in_=xt[:],
                        in_offset=bass.IndirectOffsetOnAxis(ap=ix[:, G + j:G + j + 1],
                                                            axis=0))
                cur["xa"], cur["xb"], cur["ix"] = xa, xb, ix

            xo8 = None
            ob8 = None

            for w in range(nw):
                # ---- DVE mul fused over group-runs; onehot per chunk ----
                prodw = wpool.tile([128, TBW * D], f32, tag="prodw")
                ohw = ohpool.tile([128, TBW * WE], f32, tag="oh")
                c = 0
                while c < TBW:
                    ch = w * TBW + c
                    g, j = divmod(ch, G)
                    if cur["g"] != g:
                        load_group(g)
                        cur["g"] = g
                    xa, xb, ix = cur["xa"], cur["xb"], cur["ix"]
                    run = min(TBW - c, G - j)
                    nc.vector.tensor_mul(
                        out=prodw[:, c * D:(c + run) * D],
                        in0=xa[:, j * D:(j + run) * D],
                        in1=xb[:, j * D:(j + run) * D])
                    ar = ix[:, 2 * G + j:2 * G + j + run].bitcast(f32)
                    ar_b = bass.AP(ar.tensor, ar.offset,
                                   [ar.ap[0], ar.ap[1], (0, WE)])
                    io = iota_sb[:]
                    io_b = bass.AP(io.tensor, io.offset,
                                   [io.ap[0], (0, run), io.ap[1]])
                    oh_out = ohw[:, c * WE:(c + run) * WE]
                    nc.vector.tensor_tensor(
                        out=oh_out.rearrange("p (c e) -> p c e", c=run),
                        in0=ar_b, in1=io_b, op=mybir.AluOpType.is_equal)
                    c += run

                # ---- transposes + W_msg + relu, fused MS chunks at a time --
                msbs = []
                for c0 in range(0, TBW, MS):
                    ms = min(MS, TBW - c0)
                    prodT_ps = pstpool.tile([D, MS * 128], f32, space="PSUM",
                                            tag="prodT")
                    for cc in range(ms):
                        nc.tensor.transpose(
                            out=prodT_ps[:, cc * 128:(cc + 1) * 128],
                            in_=prodw[:, (c0 + cc) * D:(c0 + cc + 1) * D],
                            identity=ident[:])
                    prodT = wpool.tile([D, MS * 128], f32, tag="prodTs")
                    nc.scalar.activation(out=prodT[:, :ms * 128],
                                         in_=prodT_ps[:, :ms * 128],
                                         func=mybir.ActivationFunctionType.Copy)
                    m_ps = psmpool.tile([128, MS * D], f32, space="PSUM",
                                        tag="mps")
                    for cc in range(ms):
                        nc.tensor.matmul(out=m_ps[:, cc * D:(cc + 1) * D],
                                         lhsT=prodT[:, cc * 128:(cc + 1) * 128],
                                         rhs=wmsg_sb[:], start=True, stop=True)
                    m_sb = wpool.tile([128, MS * D], f32, tag="msb")
                    if (w + c0) % 2 == 0:
                        nc.scalar.activation(
                            out=m_sb[:, :ms * D], in_=m_ps[:, :ms * D],
                            func=mybir.ActivationFunctionType.Relu)
                    else:
                        nc.vector.tensor_scalar(
                            out=m_sb[:, :ms * D], in0=m_ps[:, :ms * D],
                            scalar1=0.0, scalar2=None, op0=mybir.AluOpType.max)
                    msbs.append((c0, ms, m_sb))

                # ---- scatter: 4 matmuls per chunk into one [64,512] bank ---
                aggT4 = paggpool.tile([D, WE], f32, space="PSUM", tag="aggT4")
                for (c0, ms, m_sb) in msbs:
                    for cc in range(ms):
                        c = c0 + cc
                        nc.tensor.matmul(
                            out=aggT4[:],
                            lhsT=m_sb[:, cc * D:(cc + 1) * D],
                            rhs=ohw[:, c * WE:(c + 1) * WE],
                            start=(c == 0), stop=(c == TBW - 1))

                # ---- window epilogue (4 blocks) ----
                aggT4_sb = wpool.tile([D, WE], f32, tag="aggTs")
                nc.vector.tensor_copy(out=aggT4_sb[:], in_=aggT4[:])
                for q in range(W):
                    b = w * W + q
                    bb = b % EB
                    if bb == 0:
                        nb = min(EB, nblk - b)
                        xo8 = opool.tile([128, EB * D], f32, tag="xo8")
                        nc.sync.dma_start(out=xo8[:, :nb * D],
                                          in_=xown[:, b:b + nb])
                        ob8 = opool.tile([128, EB * D], f32, tag="ob8")
                    upd_ps = pupdpool.tile([128, D], f32, space="PSUM",
                                           tag="upd")
                    nc.tensor.matmul(out=upd_ps[:],
                                     lhsT=aggT4_sb[:, q * 128:(q + 1) * 128],
                                     rhs=wupd_sb[:], start=True, stop=False)
                    nc.tensor.matmul(out=upd_ps[:], lhsT=ident[:],
                                     rhs=xo8[:, bb * D:(bb + 1) * D],
                                     start=False, stop=True)
                    nc.scalar.activation(
                        out=ob8[:, bb * D:(bb + 1) * D], in_=upd_ps[:],
                        func=mybir.ActivationFunctionType.Relu)
                    if bb == EB - 1 or b == nblk - 1:
                        nb = bb + 1
                        b0 = b - bb
                        nc.sync.dma_start(out=out[:, b0:b0 + nb],
                                          in_=ob8[:, :nb * D])

    nc.compile()
    return nc


# ------------------------------------------------------------------ helpers
def make_in_maps(inputs, pre, nw, ngrp):
    nblk = nw * W
    xt32 = build_xtable(inputs["edge_feat"], inputs["relation_emb"],
                        inputs["edge_rel"])
    xos = build_xown(xt32, nblk)
    iotaw = np.tile(np.arange(WE, dtype=np.float32), (128, 1))
    in_maps = []
    for k in range(NCORES):
        in_maps.append({
            "xt": xt32,
            "xown": xos[k],
            "wmsg": np.asarray(inputs["W_msg"], np.float32),
            "wupd": np.asarray(inputs["W_upd"], np.float32),
            "iotaw": iotaw,
            "idx": pre[k]["idx"][:ngrp],
        })
    return in_maps


def unshard_out(core_outs, nblk):
    """core_outs[k]: [128, nblk, D] partition-major -> full [E, D]."""
    full = np.empty((E, D), np.float32)
    for k in range(NCORES):
        pm = np.asarray(core_outs[k])
        rows = pm.transpose(1, 0, 2).reshape(-1, D)
        n = min(EPC, E - k * EPC)
        full[k * EPC:k * EPC + n] = rows[:n]
    return full


def run_full(inputs, nw=NW):
    from concourse.bass_utils import run_bass_kernel_spmd
    pre, TBW, NGRP = host_preprocess(inputs["edge_rel"], inputs["edge_ab"],
                                     inputs["edge_bc"], inputs["edge_ac"])
    ngrp = -(-nw * TBW // G) if nw < NW else NGRP
    import time as _time
    t0 = _time.time()
    nc = build_bass(TBW, nw, ngrp)
    print(f"[build+compile {_time.time()-t0:.1f}s TBW={TBW}]", flush=True)
    in_maps = make_in_maps(inputs, pre, nw, ngrp)
    t0 = _time.time()
    res = run_bass_kernel_spmd(nc, in_maps, core_ids=list(range(NCORES)))
    print(f"[run1 {_time.time()-t0:.1f}s]", flush=True)
    outs = [np.asarray(res.results[k]["out"]) for k in range(NCORES)]
    return unshard_out(outs, nw * W)


# ------------------------------------------------------------------ entry
def kernel(**inputs):
    """Full unsharded inputs -> full [E, D] output (8-core SPMD)."""
    out = run_full(inputs, nw=NW)
    return out.astype(np.float32)


# revision 4
# speedup vs baseline: 1.0648x; 1.0103x over previous
"""Trainium2 Bass kernel v5 for the LogicMessagePassingNetwork problem.

Reference computation (E=1M edges, T=2M triangles, R=50, D=64):
    x   = edge_feat + relation_emb[edge_rel]                      # [E, D]
    m   = relu((x[edge_ab] * x[edge_bc]) @ W_msg)                 # [T, D]
    agg = segment_sum(m, edge_ac, E)                              # [E, D]
    out = relu(x + agg @ W_upd)                                   # [E, D]

v5 = v4 plus W=4-block scatter windows: triangles are bucketed per
512-edge window instead of per 128-edge block, cutting slot padding from
~46% to ~12% and with it the number of [128,1]-offset indirect gathers
(the SWDGE ~1us/call serial bottleneck). Each chunk scatters via 4
matmuls against a [128, 512] one-hot into a single-bank [64, 512] PSUM
accumulator holding all 4 blocks of the window.
"""
import numpy as np

E = 1_000_000
T = 2_000_000
R = 50
D = 64
NCORES = 8
EPC = E // NCORES          # edges per core
BLK = 128                  # output edges per block
W = 4                      # blocks per scatter window
WE = W * BLK               # edges per window (512)
NW = (EPC + WE - 1) // WE              # 245 windows/core
NBLK = NW * W                          # 980 blocks/core (padded)
EPAD = NBLK * BLK                      # padded edges/core (125440)
G = 32                     # chunks per idx-load group
EB = 8                     # blocks per epilogue batch (= 2 windows)
ZROW = E                   # index of the all-zero row in the x table
XROWS = E + 1
MS = 3                     # chunks fused per prodT/m_ps PSUM tile


# ----------------------------------------------------------------- host prep
def host_preprocess(edge_rel, edge_ab, edge_bc, edge_ac, tb_override=None):
    """Index-space preprocessing. Returns per-core index arrays + TBW + ngrp."""
    ab = np.asarray(edge_ab).astype(np.int64)
    bc = np.asarray(edge_bc).astype(np.int64)
    ac = np.asarray(edge_ac).astype(np.int64)

    order = np.argsort(ac, kind="stable")
    ab_s, bc_s, ac_s = ab[order], bc[order], ac[order]

    per_core = []
    max_cnt = 0
    for k in range(NCORES):
        lo, hi = np.searchsorted(ac_s, [k * EPC, (k + 1) * EPC])
        c_ab, c_bc, c_ac = ab_s[lo:hi], bc_s[lo:hi], ac_s[lo:hi] - k * EPC
        ccnt = np.bincount(c_ac // WE, minlength=NW)
        max_cnt = max(max_cnt, int(ccnt.max()) if len(ccnt) else 0)
        per_core.append((c_ab, c_bc, c_ac, ccnt))

    TBW = tb_override or -(-max_cnt // 128)     # chunks per window
    NCHUNK = NW * TBW
    NGRP = -(-NCHUNK // G)
    NT = NGRP * G * 128                         # padded triangle slots/core

    outs = []
    for k in range(NCORES):
        c_ab, c_bc, c_ac, ccnt = per_core[k]
        starts = np.zeros(NW, np.int64)
        starts[1:] = np.cumsum(ccnt)[:-1]
        rank = np.arange(len(c_ac)) - starts[c_ac // WE]
        slot = (c_ac // WE) * (TBW * 128) + rank

        gab = np.full(NT, ZROW, np.int32)
        gbc = np.full(NT, ZROW, np.int32)
        acrel = np.full(NT, 9999.0, np.float32)
        gab[slot] = c_ab
        gbc[slot] = c_bc
        acrel[slot] = (c_ac % WE).astype(np.float32)

        gab = gab.reshape(NGRP, G, 128).transpose(0, 2, 1)
        gbc = gbc.reshape(NGRP, G, 128).transpose(0, 2, 1)
        acrel = acrel.reshape(NGRP, G, 128).transpose(0, 2, 1)
        comb = np.concatenate([gab, gbc, acrel.view(np.int32)], axis=2)
        outs.append(dict(idx=np.ascontiguousarray(comb)))
    return outs, TBW, NGRP


def build_xtable(edge_feat, relation_emb, edge_rel):
    xt = np.zeros((XROWS, D), np.float32)
    xt[:E] = np.asarray(edge_feat, np.float32) \
        + np.asarray(relation_emb, np.float32)[np.asarray(edge_rel).astype(np.int64)]
    return xt


def build_xown(xt32, nblk):
    """Per-core own-edge x rows, partition-major [128, nblk, D]."""
    xo = []
    for k in range(NCORES):
        rows = np.zeros((EPAD, D), np.float32)
        n = min(EPC, E - k * EPC)
        rows[:n] = xt32[k * EPC:k * EPC + n]
        pm = rows.reshape(NBLK, BLK, D).transpose(1, 0, 2)[:, :nblk]
        xo.append(np.ascontiguousarray(pm))
    return xo


# ------------------------------------------------------------- device kernel
def build_bass(TBW, nw, ngrp=None):
    """nw = number of windows to emit (< NW for scaled-down testing)."""
    import concourse.bass as bass
    import concourse.bacc as bacc
    import concourse.mybir as mybir
    import concourse.tile as tile
    from concourse.masks import make_identity

    f32 = mybir.dt.float32
    i32 = mybir.dt.int32
    nblk = nw * W
    nchunk = nw * TBW
    if ngrp is None:
        ngrp = -(-nchunk // G)
    nc = bacc.Bacc(None, target_bir_lowering=False)

    xt = nc.dram_tensor("xt", [XROWS, D], f32, kind="ExternalInput")
    xown = nc.dram_tensor("xown", [128, nblk, D], f32, kind="ExternalInput")
    wmsg = nc.dram_tensor("wmsg", [D, D], f32, kind="ExternalInput")
    wupd = nc.dram_tensor("wupd", [D, D], f32, kind="ExternalInput")
    iotaw = nc.dram_tensor("iotaw", [128, WE], f32, kind="ExternalInput")
    idx = nc.dram_tensor("idx", [ngrp, 128, 3 * G], i32, kind="ExternalInput")
    out = nc.dram_tensor("out", [128, nblk, D], f32, kind="ExternalOutput")

    with tile.TileContext(nc) as tc:
        with tc.tile_pool(name="const", bufs=1) as cpool, \
             tc.tile_pool(name="gath", bufs=5) as gpool, \
             tc.tile_pool(name="idxp", bufs=5) as ipool, \
             tc.tile_pool(name="work", bufs=6) as wpool, \
             tc.tile_pool(name="ohp", bufs=2) as ohpool, \
             tc.tile_pool(name="outp", bufs=2) as opool, \
             tc.tile_pool(name="pst", bufs=2, space="PSUM") as pstpool, \
             tc.tile_pool(name="psm", bufs=2, space="PSUM") as psmpool, \
             tc.tile_pool(name="psagg", bufs=2, space="PSUM") as paggpool, \
             tc.tile_pool(name="psupd", bufs=2, space="PSUM") as pupdpool:

            wmsg_sb = cpool.tile([D, D], f32)
            nc.sync.dma_start(out=wmsg_sb[:], in_=wmsg[:])
            wupd_sb = cpool.tile([D, D], f32)
            nc.sync.dma_start(out=wupd_sb[:], in_=wupd[:])
            iota_sb = cpool.tile([128, WE], f32)
            nc.sync.dma_start(out=iota_sb[:], in_=iotaw[:])
            ident = cpool.tile([128, 128], f32)
            make_identity(nc, ident[:])

            cur = {"g": -1}

            def load_group(g):
                ix = ipool.tile([128, 3 * G], i32, tag="ix")
                nc.sync.dma_start(out=ix[:], in_=idx[g])
                xa = gpool.tile([128, G * D], f32, tag="xa")
                xb = gpool.tile([128, G * D], f32, tag="xb")
                for j in range(G):
                    nc.gpsimd.indirect_dma_start(
                        out=xa[:, j * D:(j + 1) * D], out_offset=None, in_=xt[:],
                        in_offset=bass.IndirectOffsetOnAxis(ap=ix[:, j:j + 1],
                                                            axis=0))
                    nc.gpsimd.indirect_dma_start(
                        out=xb[:, j * D:(j + 1) * D], out_offset=None, in_=xt[:],
                        in_offset=bass.IndirectOffsetOnAxis(ap=ix[:, G + j:G + j + 1],
                                                            axis=0))
                cur["xa"], cur["xb"], cur["ix"] = xa, xb, ix

            xo8 = None
            ob8 = None

            for w in range(nw):
                # ---- DVE mul fused over group-runs; onehot per chunk ----
                prodw = wpool.tile([128, TBW * D], f32, tag="prodw")
                ohw = ohpool.tile([128, TBW * WE], f32, tag="oh")
                c = 0
                while c < TBW:
                    ch = w * TBW + c
                    g, j = divmod(ch, G)
                    if cur["g"] != g:
                        load_group(g)
                        cur["g"] = g
                    xa, xb, ix = cur["xa"], cur["xb"], cur["ix"]
                    run = min(TBW - c, G - j)
                    nc.vector.tensor_mul(
                        out=prodw[:, c * D:(c + run) * D],
                        in0=xa[:, j * D:(j + run) * D],
                        in1=xb[:, j * D:(j + run) * D])
                    ar = ix[:, 2 * G + j:2 * G + j + run].bitcast(f32)
                    ar_b = bass.AP(ar.tensor, ar.offset,
                                   [ar.ap[0], ar.ap[1], (0, WE)])
                    io = iota_sb[:]
                    io_b = bass.AP(io.tensor, io.offset,
                                   [io.ap[0], (0, run), io.ap[1]])
                    oh_out = ohw[:, c * WE:(c + run) * WE]
                    nc.vector.tensor_tensor(
                        out=oh_out.rearrange("p (c e) -> p c e", c=run),
                        in0=ar_b, in1=io_b, op=mybir.AluOpType.is_equal)
                    c += run

                # ---- transposes + W_msg + relu, fused MS chunks at a time --
                msbs = []
                for c0 in range(0, TBW, MS):
                    ms = min(MS, TBW - c0)
                    prodT_ps = pstpool.tile([D, MS * 128], f32, space="PSUM",
                                            tag="prodT")
                    for cc in range(ms):
                        nc.tensor.transpose(
                            out=prodT_ps[:, cc * 128:(cc + 1) * 128],
                            in_=prodw[:, (c0 + cc) * D:(c0 + cc + 1) * D],
                            identity=ident[:])
                    prodT = wpool.tile([D, MS * 128], f32, tag="prodTs")
                    nc.scalar.activation(out=prodT[:, :ms * 128],
                                         in_=prodT_ps[:, :ms * 128],
                                         func=mybir.ActivationFunctionType.Copy)
                    m_ps = psmpool.tile([128, MS * D], f32, space="PSUM",
                                        tag="mps")
                    for cc in range(ms):
                        nc.tensor.matmul(out=m_ps[:, cc * D:(cc + 1) * D],
                                         lhsT=prodT[:, cc * 128:(cc + 1) * 128],
                                         rhs=wmsg_sb[:], start=True, stop=True)
                    m_sb = wpool.tile([128, MS * D], f32, tag="msb")
                    if (w + c0) % 2 == 0:
                        nc.scalar.activation(
                            out=m_sb[:, :ms * D], in_=m_ps[:, :ms * D],
                            func=mybir.ActivationFunctionType.Relu)
                    else:
                        nc.vector.tensor_scalar(
                            out=m_sb[:, :ms * D], in0=m_ps[:, :ms * D],
                            scalar1=0.0, scalar2=None, op0=mybir.AluOpType.max)
                    msbs.append((c0, ms, m_sb))

                # ---- scatter: 4 matmuls per chunk into one [64,512] bank ---
                aggT4 = paggpool.tile([D, WE], f32, space="PSUM", tag="aggT4")
                for (c0, ms, m_sb) in msbs:
                    for cc in range(ms):
                        c = c0 + cc
                        nc.tensor.matmul(
                            out=aggT4[:],
                            lhsT=m_sb[:, cc * D:(cc + 1) * D],
                            rhs=ohw[:, c * WE:(c + 1) * WE],
                            start=(c == 0), stop=(c == TBW - 1))

                # ---- window epilogue (4 blocks) ----
                aggT4_sb = wpool.tile([D, WE], f32, tag="aggTs")
                nc.vector.tensor_copy(out=aggT4_sb[:], in_=aggT4[:])
                for q in range(W):
                    b = w * W + q
                    bb = b % EB
                    if bb == 0:
                        nb = min(EB, nblk - b)
                        xo8 = opool.tile([128, EB * D], f32, tag="xo8")
                        nc.sync.dma_start(out=xo8[:, :nb * D],
                                          in_=xown[:, b:b + nb])
                        ob8 = opool.tile([128, EB * D], f32, tag="ob8")
                    upd_ps = pupdpool.tile([128, D], f32, space="PSUM",
                                           tag="upd")
                    nc.tensor.matmul(out=upd_ps[:],
                                     lhsT=aggT4_sb[:, q * 128:(q + 1) * 128],
                                     rhs=wupd_sb[:], start=True, stop=False)
                    nc.tensor.matmul(out=upd_ps[:], lhsT=ident[:],
                                     rhs=xo8[:, bb * D:(bb + 1) * D],
                                     start=False, stop=True)
                    nc.scalar.activation(
                        out=ob8[:, bb * D:(bb + 1) * D], in_=upd_ps[:],
                        func=mybir.ActivationFunctionType.Relu)
                    if bb == EB - 1 or b == nblk - 1:
                        nb = bb + 1
                        b0 = b - bb
                        nc.sync.dma_start(out=out[:, b0:b0 + nb],
                                          in_=ob8[:, :nb * D])

    nc.compile()
    return nc


# ------------------------------------------------------------------ helpers
def make_in_maps(inputs, pre, nw, ngrp):
    nblk = nw * W
    xt32 = build_xtable(inputs["edge_feat"], inputs["relation_emb"],
                        inputs["edge_rel"])
    xos = build_xown(xt32, nblk)
    iotaw = np.tile(np.arange(WE, dtype=np.float32), (128, 1))
    in_maps = []
    for k in range(NCORES):
        in_maps.append({
            "xt": xt32,
            "xown": xos[k],
            "wmsg": np.asarray(inputs["W_msg"], np.float32),
            "wupd": np.asarray(inputs["W_upd"], np.float32),
            "iotaw": iotaw,
            "idx": pre[k]["idx"][:ngrp],
        })
    return in_maps


def unshard_out(core_outs, nblk):
    """core_outs[k]: [128, nblk, D] partition-major -> full [E, D]."""
    full = np.empty((E, D), np.float32)
    for k in range(NCORES):
        pm = np.asarray(core_outs[k])
        rows = pm.transpose(1, 0, 2).reshape(-1, D)
        n = min(EPC, E - k * EPC)
        full[k * EPC:k * EPC + n] = rows[:n]
    return full


def run_full(inputs, nw=NW):
    from concourse.bass_utils import run_bass_kernel_spmd
    pre, TBW, NGRP = host_preprocess(inputs["edge_rel"], inputs["edge_ab"],
                                     inputs["edge_bc"], inputs["edge_ac"])
    ngrp = -(-nw * TBW // G) if nw < NW else NGRP
    import time as _time
    t0 = _time.time()
    nc = build_bass(TBW, nw, ngrp)
    print(f"[build+compile {_time.time()-t0:.1f}s TBW={TBW}]", flush=True)
    in_maps = make_in_maps(inputs, pre, nw, ngrp)
    t0 = _time.time()
    res = run_bass_kernel_spmd(nc, in_maps, core_ids=list(range(NCORES)))
    print(f"[run1 {_time.time()-t0:.1f}s]", flush=True)
    outs = [np.asarray(res.results[k]["out"]) for k in range(NCORES)]
    return unshard_out(outs, nw * W)


# ------------------------------------------------------------------ entry
def kernel(**inputs):
    """Full unsharded inputs -> full [E, D] output (8-core SPMD)."""
    out = run_full(inputs, nw=NW)
    return out.astype(np.float32)
